# revision 21
# baseline (speedup 1.0000x reference)
"""Cross-attention (4 modalities, 12 pairwise attentions) on 8 TRN2 NeuronCores.

Sharding: head-parallel. H=8 heads, one head per core. Each core computes, for
its head h: q/k/v for all 4 modalities, all 12 pairwise attentions (x2 batches),
and the partial output projection using Wproj[:, h*D:(h+1)*D]. Host sums the 8
partial outputs and adds 3*bproj.

Kernel math per core (head h), per modality pair (i, j), batch b:
  S.T[m, n] = sum_d k_j[m, d] * q_i[n, d]            (q pre-scaled by D^-0.5)
  E = exp(S.T)                                        (no max-subtract: |S|<~3)
  O'[0:64, n] = sum_m v_j[m, :] * E[m, n]  (unnormalized O.T)
  O'[64, n]   = sum_m E[m, n]              (softmax denominator, via ones col)
  OT = O'[0:64] / O'[64]                   (normalized attention output, transposed)
  out[i] += OT.T @ WprojT_slice            (3 j's accumulated in PSUM, K-stacked)

Performance notes (measured on HW via chained wall-clock differencing,
12 sequential NEFF executions per dispatch to beat the ~2ms axon noise):
  - fp32r matmuls with K=64 run ~1.6x slower per moving column than K=128, so
    attention/projection matmuls run at K=128 with zero pad rows. q and k are
    produced by ONE packed stationary per (modality, cc): cols 0:63 = wq*SCALE,
    cols 64:127 = wk (halves QKV matmul columns and psum traffic vs separate
    zero-padded q/k stationaries; measured -85us/iter). qT/kT pad rows 64:127
    are zeroed once via DMA.
  - Offloading exp tiles to DVE (Schraudolph int16->bf16) and bf16 O' matmuls
    were both tried and measured SLOWER (+113us/iter): the kernel is not
    ScalarE-exp-bound at steady state, and bf16 moving operands regress PE.
  - reciprocal_approx_fast returns garbage on HW in this toolchain (passes
    CoreSim and the compiler); keep nc.vector.reciprocal.
  - Projection of a finished group is spread across the next j-block's mc loop
    so the normalize chain (DVE recip -> gpsimd bcast -> DVE mul) never stalls
    PE.
"""

import functools

import ml_dtypes
import numpy as np

import concourse.mybir as mybir
import concourse.tile as tile
from concourse import bacc
from concourse.bass_utils import run_bass_kernel_spmd

B, N, C, H = 2, 1024, 512, 8
D = C // H          # 64
NT = B * N          # 2048 flattened tokens per modality
SCALE = D ** -0.5
NCORES = 8
NMOD = 4
F32 = mybir.dt.float32
F32R = mybir.dt.float32r
EXP = mybir.ActivationFunctionType.Exp

# Emission schedule: ("qkv", i) stages interleaved with ("att", i, b, j)
# blocks. Constraints honored:
#   - ("att", i, b, j) comes after ("qkv", i) and ("qkv", j)
#   - at most 6 groups (i, b) have attention started but projection not yet
#     emitted (ot2 pool has 6 slots)
#   - at most one group finishes its 3rd j between consecutive att blocks
#     (its projection is spread over the following block)
SCHED = [
    ("qkv", 0), ("qkv", 1),
    ("att", 0, 0, 1), ("att", 0, 1, 1), ("att", 1, 0, 0), ("att", 1, 1, 0),
    ("qkv", 2),
    ("att", 2, 0, 0), ("att", 2, 1, 0),
    ("att", 0, 0, 2), ("att", 0, 1, 2), ("att", 1, 0, 2), ("att", 1, 1, 2),
    ("att", 2, 0, 1), ("att", 2, 1, 1),
    ("qkv", 3),
    ("att", 0, 0, 3), ("att", 3, 0, 0), ("att", 0, 1, 3), ("att", 3, 0, 1),
    ("att", 1, 0, 3), ("att", 3, 0, 2), ("att", 1, 1, 3), ("att", 3, 1, 0),
    ("att", 2, 0, 3), ("att", 3, 1, 1), ("att", 2, 1, 3), ("att", 3, 1, 2),
]


def build_program(repeat=1, variant="full"):
    """Build the per-core Bass program. repeat>1 duplicates the whole compute
    (same inputs/outputs) for wall-clock differencing: HW exec time per
    iteration = (t(R) - t(1)) / (R - 1), which cancels dispatch overhead.
    """
    dma_only = variant == "dmaonly"
    mm_bench = variant.startswith("mm")
    nc = bacc.Bacc("TRN2", target_bir_lowering=False, debug=False,
                   enable_asserts=False)

    # DRAM I/O. Host pre-packs weight layouts to match SBUF layouts exactly.
    mT_d = nc.dram_tensor("mT", [NMOD, 4, 128, NT], F32R,
                          kind="ExternalInput").ap()
    wqk_d = nc.dram_tensor("wqk", [128, NMOD, 4, 128], F32R,
                           kind="ExternalInput").ap()
    wv_d = nc.dram_tensor("wv", [128, NMOD, 4, D], F32R,
                          kind="ExternalInput").ap()
    wp_d = nc.dram_tensor("wp", [128, NMOD, C], F32R,
                          kind="ExternalInput").ap()
    ones_d = nc.dram_tensor("ones", [128, NMOD, 16, 1], F32R,
                            kind="ExternalInput").ap()
    zeros_d = nc.dram_tensor("zeros", [64, NT], F32R,
                             kind="ExternalInput").ap()
    out_d = nc.dram_tensor("out", [NMOD, NT, C], F32, kind="ExternalOutput").ap()

    with tile.TileContext(nc) as tc:
        with (
            tc.tile_pool(name="persist", bufs=1) as persist,
            tc.tile_pool(name="mpool", bufs=4) as mpool,
            tc.tile_pool(name="expool", bufs=2) as expool,
            tc.tile_pool(name="otpool", bufs=6) as otpool,
            tc.tile_pool(name="zpool", bufs=2) as zpool,
            tc.tile_pool(name="obpool", bufs=2) as obpool,
            tc.tile_pool(name="ps_st", bufs=2, space="PSUM") as ps_st,
            tc.tile_pool(name="ps_oa", bufs=2, space="PSUM") as ps_oa,
        ):
            # Persistent SBUF tensors. qT/kT rows 64:127 stay zero (written by
            # the zero-padded QKV copies) -> all attention matmuls are K=128.
            qT = persist.tile([128, NMOD, NT], F32R, name="qT")
            kT = persist.tile([128, NMOD, NT], F32R, name="kT")
            vb = persist.tile([128, NMOD, 16, D + 1], F32R, name="vb")
            wqks = persist.tile([128, NMOD, 4, 128], F32R, name="wqks")
            wvs = persist.tile([128, NMOD, 4, D], F32R, name="wvs")
            wps = persist.tile([128, NMOD, C], F32R, name="wps")
            # ot1 slots: rows 64:127 zeroed once so projection j3 runs K=128
            ot1ab = [persist.tile([128, N], F32R, name=f"ot1{s}")
                     for s in "AB"]

            if mm_bench:
                _emit_mm_bench(nc, tc, repeat, variant, wqks, wps, wqk_d, wp_d)
                nc.compile()
                return nc

            for t in ot1ab:
                nc.sync.dma_start(out=t[64:128, :], in_=zeros_d[:, 0:N])
            # qT/kT rows 64:127 are read by every attention matmul (K=128)
            # but only rows 0:63 are written per rep; zero the pad rows once
            # via DMA, hidden under the initial input DMAs.
            for i in range(NMOD):
                nc.sync.dma_start(out=qT[64:128, i, :], in_=zeros_d[:])
                nc.sync.dma_start(out=kT[64:128, i, :], in_=zeros_d[:])

            for rep in range(repeat):
                _r = f"r{rep}_" if repeat > 1 else ""

                if dma_only:
                    for i in range(NMOD):
                        nc.sync.dma_start(out=wqks[:, i], in_=wqk_d[:, i])
                        for cc in range(4):
                            mt = mpool.tile([128, NT], F32R, tag="mt",
                                            name=f"{_r}dmt_{i}_{cc}")
                            nc.sync.dma_start(out=mt[:], in_=mT_d[i, cc])
                        nc.sync.dma_start(out=wvs[:, i], in_=wv_d[:, i])
                    nc.sync.dma_start(out=wps[:], in_=wp_d[:])
                    for c in range(64):
                        ob = obpool.tile([128, 512], F32, tag="ob",
                                         name=f"{_r}dob_{c}")
                        nc.vector.memset(ob[:], 0.125)
                        nc.sync.dma_start(
                            out=out_d[c // 16, (c % 16) * 128:
                                      (c % 16 + 1) * 128, :],
                            in_=ob[:])
                    continue

                # per-group state: (ot2, ot1, n_done); pending projection
                gstate = {}
                pending = [None]

                def emit_qkv(i):
                    nc.sync.dma_start(out=wqks[:, i], in_=wqk_d[:, i])
                    mts = []
                    for cc in range(4):
                        mt = mpool.tile([128, NT], F32R, tag="mt",
                                        name=f"{_r}mt_{i}_{cc}")
                        nc.sync.dma_start(out=mt[:], in_=mT_d[i, cc])
                        mts.append(mt)
                    nc.sync.dma_start(out=wvs[:, i], in_=wv_d[:, i])
                    if i == 1:
                        nc.sync.dma_start(out=wps[:], in_=wp_d[:])
                        nc.sync.dma_start(out=vb[:, :, :, D:D + 1],
                                          in_=ones_d[:])
                    # q and k packed in one stationary: cols 0:63 = wq*SCALE,
                    # cols 64:127 = wk -> one psum per nn (halves the qk
                    # matmul columns); pad rows 64:127 of qT/kT zeroed once.
                    for nn in range(4):
                        nsl = slice(nn * 512, (nn + 1) * 512)
                        qk_ps = ps_st.tile([128, 512], F32, tag="st",
                                           name=f"{_r}qkps_{i}_{nn}")
                        for cc in range(4):
                            nc.tensor.matmul(
                                qk_ps[:], wqks[:, i, cc, :],
                                mts[cc][:, nsl],
                                start=(cc == 0), stop=(cc == 3))
                        nc.vector.tensor_copy(qT[0:64, i, nsl],
                                              qk_ps[0:64, :])
                        nc.vector.tensor_copy(kT[0:64, i, nsl],
                                              qk_ps[64:128, :])
                    # v in natural layout: out [128 n, 64 d] per 128-chunk
                    for half in range(2):
                        v_ps = ps_st.tile([128, 512], F32, tag="st",
                                          name=f"{_r}vps_{i}_{half}")
                        for k8 in range(8):
                            nch = half * 8 + k8
                            for cc in range(4):
                                nc.tensor.matmul(
                                    v_ps[:, k8 * D:(k8 + 1) * D],
                                    mts[cc][:, nch * 128:(nch + 1) * 128],
                                    wvs[:, i, cc, :],
                                    start=(cc == 0), stop=(cc == 3))
                        nc.vector.tensor_copy(
                            vb[:, i, half * 8:(half + 1) * 8, 0:D],
                            v_ps[:].rearrange("p (k d) -> p k d", d=D))

                def emit_proj_chunk(i, b, ot2, ot1, c8):
                    # One 128-row tile of out[i, b]; 3 j's accumulated in PSUM
                    # oa-tag slot: pp drains at DVE-copy pace without
                    # occupying the S.T double-buffer (which feeds ScalarE)
                    pp = ps_oa.tile([128, 512], F32, tag="oa",
                                    name=f"{_r}pp_{i}_{b}_{c8}")
                    csl = slice(c8 * 128, (c8 + 1) * 128)
                    nc.tensor.matmul(pp[:], ot2[:, csl], wps[:, i, :],
                                     start=True, stop=False)
                    nc.tensor.matmul(pp[:], ot1[:, csl], wps[:, i, :],
                                     start=False, stop=True)
                    ob = obpool.tile([128, 512], F32, tag="ob",
                                     name=f"{_r}ob_{i}_{b}_{c8}")
                    nc.vector.tensor_copy(ob[:], pp[:])
                    nc.sync.dma_start(
                        out=out_d[i, b * N + c8 * 128:b * N + (c8 + 1) * 128,
                                  :],
                        in_=ob[:])

                def emit_att(i, b, j):
                    g = (i, b)
                    if g not in gstate:
                        ot2 = otpool.tile([128, N], F32R, tag="ot2",
                                          name=f"{_r}ot2_{i}_{b}")
                        gstate[g] = [ot2, ot1ab[(i * B + b) % 2], 0]
                    ot2, ot1, n_done = gstate[g]
                    qTs = qT[:, i, b * N:(b + 1) * N]
                    kTs = kT[:, j, b * N:(b + 1) * N]
                    oa = ps_oa.tile([128, N], F32, tag="oa",
                                    name=f"{_r}oa_{i}_{b}_{j}")
                    for mc in range(8):
                        st = ps_st.tile([128, N], F32, tag="st",
                                        name=f"{_r}st_{i}_{b}_{j}_{mc}")
                        for nn2 in range(2):
                            nc.tensor.matmul(
                                st[:, nn2 * 512:(nn2 + 1) * 512],
                                kTs[:, mc * 128:(mc + 1) * 128],
                                qTs[:, nn2 * 512:(nn2 + 1) * 512],
                                start=True, stop=True)
                        ex = expool.tile([128, N], F32R, tag="ex",
                                         name=f"{_r}ex_{i}_{b}_{j}_{mc}")
                        nc.scalar.activation(ex[:], st[:], EXP)
                        for nn2 in range(2):
                            nc.tensor.matmul(
                                oa[0:D + 1, nn2 * 512:(nn2 + 1) * 512],
                                vb[:, j, b * 8 + mc, :],
                                ex[:, nn2 * 512:(nn2 + 1) * 512],
                                start=(mc == 0), stop=(mc == 7))
                        # spread the pending group's projection over this loop
                        if pending[0] is not None:
                            emit_proj_chunk(*pending[0], mc)
                            if mc == 7:
                                pending[0] = None
                    # normalize: OT = O'[0:64] * (1 / O'[64])
                    rr = zpool.tile([1, N], F32, tag="rr",
                                    name=f"{_r}rr_{i}_{b}_{j}")
                    nc.vector.reciprocal(rr[:], oa[D:D + 1, :])
                    rb = zpool.tile([64, N], F32, tag="rb",
                                    name=f"{_r}rb_{i}_{b}_{j}")
                    nc.gpsimd.partition_broadcast(rb[:], rr[:])
                    if n_done < 2:
                        dest = ot2[n_done * 64:(n_done + 1) * 64, :]
                    else:
                        dest = ot1[0:64, :]
                    nc.vector.tensor_mul(dest, oa[0:D, :], rb[:])
                    gstate[g][2] = n_done + 1
                    if n_done + 1 == 3:
                        assert pending[0] is None
                        pending[0] = (i, b, ot2, ot1)
                        del gstate[g]

                for step in SCHED:
                    if step[0] == "qkv":
                        emit_qkv(step[1])
                    else:
                        emit_att(*step[1:])
                assert pending[0] is not None
                for c8 in range(8):
                    emit_proj_chunk(*pending[0], c8)
                pending[0] = None

    nc.compile()
    return nc


def _emit_mm_bench(nc, tc, repeat, variant, wqks, wps, wqk_d, wp_d):
    """PE micro-bench: 2000*repeat back-to-back fp32r matmuls, no deps."""
    for i in range(NMOD):
        nc.sync.dma_start(out=wqks[:, i], in_=wqk_d[:, i])
    nc.sync.dma_start(out=wps[:], in_=wp_d[:])
    with tc.tile_pool(name="ps_mm", bufs=2, space="PSUM") as ps_mm:
        for k in range(2000 * repeat):
            ps = ps_mm.tile([128, 512], F32, tag="mm", name=f"mmps_{k}")
            if variant == "mm64":
                nc.tensor.matmul(ps[:], wqks[0:64, k % 4, k % 4, :],
                                 wps[0:64, k % 4, :], start=True, stop=True)
            else:  # mm128
                nc.tensor.matmul(ps[:], wqks[:, k % 4, k % 4, :],
                                 wps[:, k % 4, :], start=True, stop=True)


@functools.lru_cache(maxsize=8)
def _cached_program(repeat=1, variant="full"):
    return build_program(repeat, variant)


def round_fp32r(x):
    """Round fp32 to fp32r (11 explicit mantissa bits, round-to-nearest-even).

    Matches neuron_dtypes.static_cast_fp32_to_fp32r bit-exactly for normal
    values. The PE consumes fp32r operands; DMA cannot round, so inputs are
    pre-rounded on the host.
    """
    u = np.ascontiguousarray(x, dtype=np.float32).view(np.uint32)
    r = (u + np.uint32(0x7FF) + ((u >> np.uint32(12)) & np.uint32(1))) \
        & np.uint32(0xFFFFF000)
    return r.view(np.float32)


def prep_inputs(inputs):
    """Build per-core input maps from the full problem inputs."""
    m = np.stack([np.asarray(inputs[f"m{i+1}"], dtype=np.float32)
                  for i in range(NMOD)])                       # [4, B, N, C]
    Wqkv = np.asarray(inputs["Wqkv"], dtype=np.float32)        # [4, 3C, C]
    Wproj = np.asarray(inputs["Wproj"], dtype=np.float32)      # [4, C, C]

    mt = round_fp32r(np.ascontiguousarray(
        m.reshape(NMOD, NT, C).transpose(0, 2, 1).reshape(NMOD, 4, 128, NT)))

    in_maps = []
    for h in range(NCORES):
        hsl = slice(h * D, (h + 1) * D)
        wq = Wqkv[:, 0 * C:1 * C, :][:, hsl, :] * SCALE        # [4, D, C]
        wk = Wqkv[:, 1 * C:2 * C, :][:, hsl, :]
        wv = Wqkv[:, 2 * C:3 * C, :][:, hsl, :]
        # q and k packed into one 128-col stationary: cols 0:63 = wq*SCALE,
        # cols 64:127 = wk; qT/kT pad rows are zeroed in-kernel instead.
        wqkz = np.concatenate([wq, wk], axis=1)                # [i, 2D, C]
        # [p, i, cc, col] = wqkz[i, col, cc*128+p]
        wqks = round_fp32r(np.ascontiguousarray(
            wqkz.reshape(NMOD, 2 * D, 4, 128).transpose(3, 0, 2, 1)))
        wvs = round_fp32r(np.ascontiguousarray(
            wv.reshape(NMOD, D, 4, 128).transpose(3, 0, 2, 1)))
        wpt = Wproj[:, :, hsl].transpose(0, 2, 1)              # [4, D, C]
        wps = round_fp32r(np.ascontiguousarray(
            np.concatenate([wpt, wpt], axis=1).transpose(1, 0, 2)))
        in_maps.append({"mT": mt, "wqk": wqks, "wv": wvs, "wp": wps,
                        "ones": np.ones((128, NMOD, 16, 1), np.float32),
                        "zeros": np.zeros((64, NT), np.float32)})
    return in_maps


def run_cores(inputs, trace=False, repeat=1, **kwargs):
    nc = _cached_program(repeat)
    in_maps = prep_inputs(inputs)
    res = run_bass_kernel_spmd(nc, in_maps, core_ids=list(range(NCORES)),
                               trace=trace, **kwargs)
    return res


def finalize(results, bproj):
    acc = np.zeros((NMOD, NT, C), dtype=np.float32)
    for r in results:
        acc += r["out"]
    acc = acc.reshape(NMOD, B, N, C)
    acc += 3.0 * np.asarray(bproj, dtype=np.float32)[:, None, None, :]
    return acc


def kernel(m1, m2, m3, m4, Wqkv, Wproj, bproj):
    inputs = {"m1": m1, "m2": m2, "m3": m3, "m4": m4,
              "Wqkv": Wqkv, "Wproj": Wproj, "bproj": bproj}
    res = run_cores(inputs)
    return finalize(res.results, bproj)

